# revision 8
# baseline (speedup 1.0000x reference)
"""Concordance-index loss on Trainium2 (8 NeuronCores, raw Bass) — v8.

DVE-only staircase on top of v6's decomposition: i sorted by tm on
partitions, j sorted by u on the free dim; comp[:, j] is a prefix of
length p_j of the sorted-i order, so each (i-tile, j-column) is
all-zeros, all-ones, or in the single boundary window.

v8 collapses the whole PE/PSUM reduction pipeline into the DVE's
`accum_out` port: every compare instruction also emits its per-partition
free-dim sum into a [128, 32] f32 accumulator.  Per live tile t:

  (c) rgt+accum over the all-ones region [hi_t, JW):
        acc[:, t] = sum_j (r_j < re_i)
  (a) rgt over the boundary window [lo_t, hi_t) into scratch
  (b) fused scalar_tensor_tensor over the window:
        acc[:, 16+t] = sum_j (u_j > tm_i) * (r_j < re_i)

No PSUM, no matmuls, no PSUM evacuation, no DVE<->PE ping-pong, and no
priming run.  Sync graph: DMA-A(r row + scalars) -> DVE, DMA-B(u row)
-> DVE window pass (overlapped with the all-ones pass), DVE -> DMA-out
of the 128B/partition accumulator.  `total` is exact host rank math and
`tied` is host-enumerated (as in v6); conc = sum(acc) on host.

SPMD strided shards exactly as v6: core c = (iq, jh) owns sorted-i
positions t*512 + 4*p + iq and sorted-j positions 2*k + jh, so the
per-tile windows are near identical across cores and the compiled
program uses their union (correct for every core).
"""

import numpy as np

N = 8192
NCORES = 8
P = 128
NIQ = 2                     # i stride
NJH = 4                     # j stride
IBLK = N // NIQ             # 4096 i's per core
IT = IBLK // P              # 32 i partition-tiles per core
IBAND = P * NIQ             # 256: global sorted-i band per tile
JW = N // NJH               # 2048 j's per core (free dim)
NCHUNK = 4                  # rj DMA chunks (high columns first)
WSLOT = 1024                # window scratch width (max union window)
NBA = 2 * JW + 2 * IT * 4   # bytes/partition in xina: rj row + tmi|rei f32
NBB = 2 * JW                # bytes/partition in xinb: uj row
NACC = 2 * IT               # acc columns: t -> all-ones, IT+t -> window

_CACHE = {}


def _align_windows(windows):
    """(lo, hi) -> (lo&~3, min(JW, (hi+3)&~3)): 8B-aligned for DVE 4x
    mode; the extra columns are exactly-handled by the window compare."""
    out = []
    for lo, hi in windows:
        if lo >= JW:
            out.append((JW, JW))
            continue
        lo_a = lo & ~3
        hi_a = min(JW, (hi + 3) & ~3)
        assert hi_a - lo_a <= WSLOT, (lo, hi)
        out.append((lo_a, hi_a))
    return out


def _build_nc(windows, repeat=1):
    """windows: per tile t, (lo_t, hi_t): columns [0, lo_t) are all-zeros,
    [lo_t, hi_t) boundary (elementwise), [hi_t, JW) all-ones."""
    import concourse.bass as bass
    from concourse import mybir

    dt = mybir.dt
    Alu = mybir.AluOpType
    wal = _align_windows(windows)
    live = [t for t in range(IT) if wal[t][0] < JW]

    # DMA-A is split into NCHUNK column chunks, high columns first:
    # a tile's all-ones pass starts as soon as the chunks covering
    # [hi_t, JW) have landed, so compute begins ~450ns after the first
    # (smallest) chunk instead of after the whole r row.
    bounds = [JW - k * (JW // NCHUNK) for k in range(NCHUNK + 1)]  # desc
    groups = [
        [t for t in live if wal[t][1] < JW
         and bounds[k + 1] <= wal[t][1] < bounds[k]]
        for k in range(NCHUNK)
    ]

    nc = bass.Bass()
    xina = nc.declare_dram_parameter("xina", [P, NBA], dt.uint8,
                                     isOutput=False)
    xinb = nc.declare_dram_parameter("xinb", [P, NBB], dt.uint8,
                                     isOutput=False)
    out = nc.declare_dram_parameter("out", [P, NACC], dt.float32,
                                    isOutput=True)

    with (
        nc.sbuf_tensor([P, NBA], dt.uint8) as xina_s,
        nc.sbuf_tensor([P, NBB], dt.uint8) as xinb_s,
        nc.sbuf_tensor([P, NACC], dt.float32) as acc_s,
        nc.sbuf_tensor([P, WSLOT], dt.float16) as wscr,
        nc.sbuf_tensor([P, WSLOT], dt.float16) as prod,
        nc.sbuf_tensor([P, JW], dt.float16) as scr,
        nc.semaphore() as dsa0,
        nc.semaphore() as dsa1,
        nc.semaphore() as dsa2,
        nc.semaphore() as dsa3,
        nc.semaphore() as dsb,
        nc.semaphore() as vsem,
        nc.Block() as block,
    ):
        dsa = [dsa0, dsa1, dsa2, dsa3]
        rj_row = xina_s[:, 0:2 * JW].bitcast(dt.float16)
        xf32 = xina_s[:, 2 * JW:NBA].bitcast(dt.float32)
        tmi_s = xf32[:, 0 * IT:1 * IT]
        rei_s = xf32[:, 1 * IT:2 * IT]
        uj_row = xinb_s[:].bitcast(dt.float16)

        @block.sync
        def _(s):
            # chunk 0 (highest columns) also carries the tmi/rei scalars
            s.dma_start(xina_s[:, 2 * bounds[1]:NBA],
                        xina[:, 2 * bounds[1]:NBA]).then_inc(dsa[0], 16)
            for k in range(1, NCHUNK):
                s.dma_start(
                    xina_s[:, 2 * bounds[k + 1]:2 * bounds[k]],
                    xina[:, 2 * bounds[k + 1]:2 * bounds[k]],
                ).then_inc(dsa[k], 16)
            s.dma_start(xinb_s[:], xinb[:]).then_inc(dsb, 16)
            s.wait_ge(vsem, 1)
            s.dma_start(out[:], acc_s[:]).then_inc(dsa[0], 16)

        @block.vector
        def _(v):
            v.memset(acc_s[:], 0.0)

            def one_pass():
                # all-ones regions: acc[:, t] = sum_j rgt  (j >= hi_t)
                for k in range(NCHUNK):
                    v.wait_ge(dsa[k], 16)
                    for t in groups[k]:
                        _lo, hi = wal[t]
                        # op1 is the accumulator's reduce op (sum)
                        v.tensor_scalar(
                            scr[:, hi:JW], rj_row[:, hi:JW],
                            rei_s[:, t:t + 1], None, Alu.is_lt,
                            Alu.add, accum_out=acc_s[:, t:t + 1])
                # boundary windows (need the u row from DMA-B)
                v.wait_ge(dsb, 16)
                last = None
                for t in live:
                    lo, hi = wal[t]
                    if hi > lo:
                        w = hi - lo
                        v.tensor_scalar(
                            wscr[:, 0:w], rj_row[:, lo:hi],
                            rei_s[:, t:t + 1], None, Alu.is_lt)
                        last = v.scalar_tensor_tensor(
                            prod[:, 0:w], uj_row[:, lo:hi],
                            tmi_s[:, t:t + 1], wscr[:, 0:w],
                            Alu.is_gt, Alu.mult,
                            accum_out=acc_s[:, IT + t:IT + t + 1])
                # the sem rides the last engine op's retirement: no drain
                # needed before the out-DMA (its DGE setup alone exceeds
                # the write-ack window)
                last.then_inc(vsem, 1)

            if repeat == 1:
                one_pass()
            else:
                with v.Fori(0, repeat) as _i:
                    one_pass()

    return nc


def _encode(event_indicator, event_time, estimate):
    d = np.asarray(event_indicator).reshape(-1).astype(bool)
    t = np.asarray(event_time, dtype=np.float32).reshape(-1)
    r = np.asarray(estimate, dtype=np.float32).reshape(-1)
    assert t.shape[0] == N

    tv = np.unique(t)
    trk = np.searchsorted(tv, t).astype(np.float32)
    # fp16 must represent trk and trk+0.5 exactly -> need trk+1 < 1024
    assert len(tv) + 2 < 1024, "t ranks must stay fp16-exact incl. +0.5"
    return d, t, r, trk


def _structure(event_indicator, event_time, estimate):
    """Sorted orders, encodings, exact total, and per-tile union windows."""
    d, _t, r, trk = _encode(event_indicator, event_time, estimate)

    u = (trk + np.float32(0.5) * (~d).astype(np.float32)).astype(np.float16)
    tm = np.where(d, trk, np.float32(32768.0)).astype(np.float16)

    rv = np.unique(r)
    m = len(rv)
    assert m + 1024 < 31744, "r rank embedding must stay in normal fp16 range"
    emb = (np.arange(m, dtype=np.uint16) + np.uint16(1024)).view(np.float16)
    r_e = emb[np.searchsorted(rv, r)]

    iord = np.argsort(tm.astype(np.float32), kind="stable")  # i by tm asc
    jord = np.argsort(u.astype(np.float32), kind="stable")   # j by u asc
    tms = tm[iord].astype(np.float32)
    us = u[jord].astype(np.float32)

    # p_j = #{i: tm_i < u_j}: prefix length in sorted-i order (exact ints)
    pj_sorted = np.searchsorted(tms, us, side="left")
    total = float(pj_sorted.sum())

    # union windows over cores: core (iq, jh) takes j positions 2k+jh; its
    # tile t spans global-i band [t*512, (t+1)*512).  Column k is all-zeros
    # for tile t iff pj <= t*512, all-ones iff pj >= (t+1)*512.
    windows = []
    for t in range(IT):
        lo_u, hi_u = JW, 0
        for jh in range(NJH):
            pj_loc = pj_sorted[jh::NJH]
            lo = int(np.searchsorted(pj_loc, t * IBAND, side="right"))
            hi = int(np.searchsorted(pj_loc, (t + 1) * IBAND, side="left"))
            lo_u, hi_u = min(lo_u, lo), max(hi_u, hi)
        windows.append((lo_u, hi_u))
    return d, r, trk, u, tm, r_e, iord, jord, total, tuple(windows)


def _prep_inputs(event_indicator, event_time, estimate):
    (_d, _r, _trk, u, tm, r_e, iord, jord, _total,
     _windows) = _structure(event_indicator, event_time, estimate)

    in_maps = []
    for c in range(NCORES):
        iq, jh = divmod(c, NJH)
        li = np.arange(IBLK)
        isel = iord[(li // P) * IBAND + (li % P) * NIQ + iq]
        jsel = jord[jh::NJH]
        tmi = np.ascontiguousarray(
            tm[isel].astype(np.float32).reshape(IT, P).T)
        rei = np.ascontiguousarray(
            r_e[isel].astype(np.float32).reshape(IT, P).T)
        b32 = np.ascontiguousarray(
            np.concatenate([tmi, rei], axis=1)).view(np.uint8).reshape(P, -1)
        rj_b = np.ascontiguousarray(
            np.broadcast_to(r_e[jsel][None, :], (P, JW))).view(np.uint8)
        uj_b = np.ascontiguousarray(
            np.broadcast_to(u[jsel][None, :], (P, JW))).view(np.uint8)
        in_maps.append({
            "xina": np.ascontiguousarray(np.concatenate([rj_b, b32], axis=1)),
            "xinb": uj_b,
        })
    return in_maps


def _tied_host(event_indicator, event_time, estimate):
    """Exact tied_risk count (see kernel_v2 docstring)."""
    d, _t, r, trk = _encode(event_indicator, event_time, estimate)

    thr = np.float32(1e-8)
    order = np.argsort(r, kind="stable")
    rs = r[order]
    lo = np.zeros(N, dtype=np.int64)
    hi = np.zeros(N, dtype=np.int64)
    p = 0
    for k in range(N):
        while np.abs(rs[k] - rs[p]) > thr:
            p += 1
        lo[k] = p
    p = N - 1
    for k in range(N - 1, -1, -1):
        while np.abs(rs[k] - rs[p]) > thr:
            p -= 1
        hi[k] = p

    cnt = hi - lo + 1
    T = int(cnt.sum())
    K = np.repeat(np.arange(N, dtype=np.int64), cnt)
    offs = np.concatenate(([0], np.cumsum(cnt)[:-1]))
    Ppos = np.arange(T, dtype=np.int64) - np.repeat(offs, cnt) + np.repeat(lo, cnt)
    i_idx = order[K]
    j_idx = order[Ppos]
    comp = d[i_idx] & (
        (trk[i_idx] < trk[j_idx])
        | ((trk[i_idx] == trk[j_idx]) & (~d[j_idx]))
    )
    return float(comp.sum())


def _finish(results, total, tied):
    conc = np.float64(0.0)
    for res in results:
        conc += res["out"].astype(np.float64).sum()
    disc = total - conc - tied
    loss = (disc + 0.5 * tied) / (disc + conc + tied + 1e-7)
    return np.asarray(1.0 - loss, dtype=np.float32)


def kernel(event_indicator, event_time, estimate):
    from concourse.bass_utils import run_bass_kernel_spmd

    st = _structure(event_indicator, event_time, estimate)
    total, windows = st[8], st[9]
    in_maps = _prep_inputs(event_indicator, event_time, estimate)
    tied = _tied_host(event_indicator, event_time, estimate)

    if _CACHE.get("windows") != windows:
        _CACHE["nc"] = _build_nc(windows)
        _CACHE["windows"] = windows
    nc = _CACHE["nc"]
    out = run_bass_kernel_spmd(nc, in_maps, core_ids=list(range(NCORES)))
    return _finish(out.results, total, tied)


# revision 27
# speedup vs baseline: 2.4961x; 2.4961x over previous
"""Concordance-index loss on Trainium2 (8 NeuronCores, raw Bass) — v9.

Staircase decomposition (i sorted by tm on partitions, j sorted by u on
the free dim; comp[:, j] is a prefix of length p_j of the sorted-i
order) with HW-calibrated engine assignment:

  - DVE runs only fast-mode ops (~0.17 ns/col measured): per live tile
    a plain rgt = (r_j < re_i) compare over [lo_t, JW) into a dedicated
    buffer, plus the boundary window's comp = (u_j > tm_i) and
    prod = min(comp, rgt).  No accum_out anywhere (measured ~5x slower
    per column on HW).
  - PE reduces everything: ones-weight matmuls accumulate every tile
    region and window product into a single [1, MMW] PSUM row
    (start=False onto a DVE-memset row; accumulation across slices is
    fine because only the grand total is needed).  Measured ~0.1
    ns/col — fully hidden behind the DVE.  Warmup matmuls on a zeroed
    scratch region ramp the PE pstate during the input DMA.
  - PE trails the DVE by ONE unit (wait csem >= u+2), so tile buffers
    are long-retired when read — no per-tile drains.

The r row is DMA'd in NCHUNK high-first column chunks so the deepest
tiles start ~1.6us after launch; the u row overlaps the rgt pass.
`total` is exact host rank math and `tied` is host-enumerated; conc is
the sum of the evacuated PSUM row ([1, MMW] f32, the only output).

SPMD strided shards exactly as v6: core c = (iq, jh) owns sorted-i
positions t*512 + 4*p + iq and sorted-j positions 2*k + jh, so the
per-tile windows are near identical across cores and the compiled
program uses their union (correct for every core).
"""

from contextlib import ExitStack

import numpy as np

N = 8192
NCORES = 8
P = 128
NIQ = 4                     # i stride (quarters)
NJH = 2                     # j stride (halves)
IBLK = N // NIQ             # 2048 i's per core
IT = IBLK // P              # 16 i partition-tiles per core
IBAND = P * NIQ             # 512: global sorted-i band per tile
JW = N // NJH               # 4096 j's per core (free dim)
NCHUNK = 4                  # rj DMA chunks (high columns first)
WSLOT = 1024                # window scratch width (max union window)
MMW = 256                   # PSUM accumulation row width
NWARM = 26                  # PE warmup matmuls (pstate ramp during DMA)
NBA = 2 * JW + 2 * IT * 4 + 8   # xina: rj row + tmi|rei f32 + ones f16
NBB = 2 * JW                # xinb: uj row

_CACHE = {}


def _align_windows(windows):
    """(lo, hi) -> (lo&~3, min(JW, (hi+3)&~3)): 8B-aligned regions; the
    extra columns are exactly-handled by the window compare."""
    out = []
    for lo, hi in windows:
        if lo >= JW:
            out.append((JW, JW))
            continue
        lo_a = lo & ~3
        hi_a = min(JW, (hi + 3) & ~3)
        assert hi_a - lo_a <= WSLOT, (lo, hi)
        out.append((lo_a, hi_a))
    return out


def _build_nc(windows, repeat=1, unit_filter=None):
    """windows: per tile t, (lo_t, hi_t): columns [0, lo_t) are all-zeros,
    [lo_t, hi_t) boundary (elementwise), [hi_t, JW) all-ones.
    unit_filter: debug-only — restrict PE accumulation to ("rgt"|"win",
    tile) units passing the predicate."""
    import concourse.bass as bass
    from concourse import mybir

    dt = mybir.dt
    Alu = mybir.AluOpType
    wal = _align_windows(windows)
    live = [t for t in range(IT) if wal[t][0] < JW]

    # rj chunk k covers columns [bounds[k+1], bounds[k]); a tile's rgt op
    # reads rj[lo_t:JW], so tile t goes in the group of its lo_t chunk.
    bounds = [JW - k * (JW // NCHUNK) for k in range(NCHUNK + 1)]  # desc
    groups = [
        [t for t in live if bounds[k + 1] <= wal[t][0] < bounds[k]]
        for k in range(NCHUNK)
    ]
    # fixed global unit order shared by DVE (producer) and PE (consumer)
    units = [("rgt", t) for k in range(NCHUNK) for t in groups[k]]
    units += [("win", t) for t in live if wal[t][1] > wal[t][0]]
    nunits = len(units)

    nc = bass.Bass()
    xina = nc.declare_dram_parameter("xina", [P, NBA], dt.uint8,
                                     isOutput=False)
    xinb = nc.declare_dram_parameter("xinb", [P, NBB], dt.uint8,
                                     isOutput=False)
    out = nc.declare_dram_parameter("out", [1, MMW], dt.float32,
                                    isOutput=True)

    with (
        nc.sbuf_tensor([P, NBA], dt.uint8) as xina_s,
        nc.sbuf_tensor([P, NBB], dt.uint8) as xinb_s,
        nc.sbuf_tensor([1, MMW], dt.float32) as evac_s,
        nc.sbuf_tensor([P, JW], dt.float16) as wscr,
        nc.psum_tensor([P, 512], dt.float32) as ps,
        nc.semaphore() as dsa0,
        nc.semaphore() as dsa1,
        nc.semaphore() as dsa2,
        nc.semaphore() as dsa3,
        nc.semaphore() as dsb,
        nc.semaphore() as wsem,
        nc.semaphore() as wsem2,
        nc.semaphore() as csem,
        nc.semaphore() as psem,
        nc.semaphore() as vsem,
        ExitStack() as stack,
        nc.Block() as block,
    ):
        dsa = [dsa0, dsa1, dsa2, dsa3]
        scr = {t: stack.enter_context(
            nc.sbuf_tensor(f"scr{t}", [P, JW], dt.float16)) for t in live}
        prod = {t: stack.enter_context(
            nc.sbuf_tensor(f"prod{t}", [P, WSLOT], dt.float16))
            for t in live}
        rj_row = xina_s[:, 0:2 * JW].bitcast(dt.float16)
        xf32 = xina_s[:, 2 * JW:2 * JW + 2 * IT * 4].bitcast(dt.float32)
        tmi_s = xf32[:, 0 * IT:1 * IT]
        rei_s = xf32[:, 1 * IT:2 * IT]
        ones_w = xina_s[:, NBA - 8:NBA - 6].bitcast(dt.float16)
        uj_row = xinb_s[:].bitcast(dt.float16)

        @block.sync
        def _(s):
            # chunk 0 (highest columns) also carries scalars + ones
            s.dma_start(xina_s[:, 2 * bounds[1]:NBA],
                        xina[:, 2 * bounds[1]:NBA]).then_inc(dsa[0], 16)
            for k in range(1, NCHUNK):
                s.dma_start(
                    xina_s[:, 2 * bounds[k + 1]:2 * bounds[k]],
                    xina[:, 2 * bounds[k + 1]:2 * bounds[k]],
                ).then_inc(dsa[k], 16)
            s.dma_start(xinb_s[:], xinb[:]).then_inc(dsb, 16)
            s.wait_ge(vsem, 1)
            s.dma_start(out[:], evac_s[:]).then_inc(dsa[0], 16)

        @block.vector
        def _(v):
            # zeroed scratch for PE warmup (weights + moving data)
            v.memset(wscr[:, 0:MMW], 0.0).then_inc(wsem, 1)

            def one_pass():
                for k in range(NCHUNK):
                    v.wait_ge(dsa[k], 16)
                    for t in groups[k]:
                        lo, _hi = wal[t]
                        v.tensor_scalar(
                            scr[t][:, lo:JW], rj_row[:, lo:JW],
                            rei_s[:, t:t + 1], None,
                            Alu.is_lt).then_inc(csem, 1)
                v.wait_ge(dsb, 16)
                v.wait_ge(wsem2, 1)
                for t in live:
                    lo, hi = wal[t]
                    if hi > lo:
                        w = hi - lo
                        # the two tensor_tensor INPUTS must share their
                        # column offset (measured: mixed-offset inputs
                        # read garbage); the output offset is free
                        v.tensor_scalar(
                            wscr[:, lo:hi], uj_row[:, lo:hi],
                            tmi_s[:, t:t + 1], None, Alu.is_gt)
                        v.tensor_tensor(
                            prod[t][:, 0:w], wscr[:, lo:hi],
                            scr[t][:, lo:hi], Alu.min).then_inc(csem, 1)
                # release the PE's one-unit lag on the final unit
                v.drain()
                v.sem_inc(csem, 1)

            if repeat == 1:
                one_pass()
            else:
                with v.Fori(0, repeat) as _i:
                    one_pass()
            # evacuate the accumulation row once the PE is done
            v.wait_ge(psem, 1)
            v.tensor_copy(evac_s[:], ps[0:1, 0:MMW]).then_inc(vsem, 1)

        @block.tensor
        def _(te):
            # pstate warmup on zeroed scratch while the input DMA runs
            te.wait_ge(wsem, 1)
            for _ in range(NWARM):
                te.matmul(ps[0:1, 256:256 + MMW], wscr[:, 0:1],
                          wscr[:, 0:MMW], start=True, stop=False,
                          skip_group_check=True)
            # zero the real accumulation row (0-weights x 0-data), twice:
            # on the literal first execution after device load a single
            # start=True write has been seen not to take effect (v6 bug)
            for k in range(2):
                mm = te.matmul(ps[0:1, 0:MMW], wscr[:, 0:1],
                               wscr[:, 0:MMW], start=True, stop=False,
                               skip_group_check=True)
            # warmup reads of wscr are done: the DVE may overwrite it
            mm.then_inc(wsem2, 1)

            def one_pass():
                for u, (kind, t) in enumerate(units):
                    # trail the DVE by one unit: its writes are long
                    # retired by the time the matmuls read them
                    te.wait_ge(csem, min(u + 2, nunits + 1))
                    if unit_filter is not None and not unit_filter(kind, t):
                        continue
                    lo, hi = wal[t]
                    if kind == "rgt":
                        c0, c1 = hi, JW
                        src, off = scr[t], 0
                    else:
                        c0, c1 = 0, hi - lo
                        src, off = prod[t], 0
                    for c in range(c0, c1, MMW):
                        w = min(MMW, c1 - c)
                        te.matmul(ps[0:1, 0:w], ones_w[:, 0:1],
                                  src[:, off + c:off + c + w],
                                  start=False, stop=False,
                                  skip_group_check=True)

            if repeat == 1:
                one_pass()
            else:
                with te.Fori(0, repeat) as _i:
                    one_pass()
            # flush in-flight PSUM writes before the DVE evacuates
            te.drain()
            te.nop().then_inc(psem, 1)

    return nc


def _encode(event_indicator, event_time, estimate):
    d = np.asarray(event_indicator).reshape(-1).astype(bool)
    t = np.asarray(event_time, dtype=np.float32).reshape(-1)
    r = np.asarray(estimate, dtype=np.float32).reshape(-1)
    assert t.shape[0] == N

    tv = np.unique(t)
    trk = np.searchsorted(tv, t).astype(np.float32)
    # fp16 must represent trk and trk+0.5 exactly -> need trk+1 < 1024
    assert len(tv) + 2 < 1024, "t ranks must stay fp16-exact incl. +0.5"
    return d, t, r, trk


def _structure(event_indicator, event_time, estimate):
    """Sorted orders, encodings, exact total, and per-tile union windows."""
    d, _t, r, trk = _encode(event_indicator, event_time, estimate)

    u = (trk + np.float32(0.5) * (~d).astype(np.float32)).astype(np.float16)
    tm = np.where(d, trk, np.float32(32768.0)).astype(np.float16)

    rv = np.unique(r)
    m = len(rv)
    assert m + 1024 < 31744, "r rank embedding must stay in normal fp16 range"
    emb = (np.arange(m, dtype=np.uint16) + np.uint16(1024)).view(np.float16)
    r_e = emb[np.searchsorted(rv, r)]

    iord = np.argsort(tm.astype(np.float32), kind="stable")  # i by tm asc
    jord = np.argsort(u.astype(np.float32), kind="stable")   # j by u asc
    tms = tm[iord].astype(np.float32)
    us = u[jord].astype(np.float32)

    # p_j = #{i: tm_i < u_j}: prefix length in sorted-i order (exact ints)
    pj_sorted = np.searchsorted(tms, us, side="left")
    total = float(pj_sorted.sum())

    # union windows over cores: core (iq, jh) takes j positions 2k+jh; its
    # tile t spans global-i band [t*512, (t+1)*512).  Column k is all-zeros
    # for tile t iff pj <= t*512, all-ones iff pj >= (t+1)*512.
    windows = []
    for t in range(IT):
        lo_u, hi_u = JW, 0
        for jh in range(NJH):
            pj_loc = pj_sorted[jh::NJH]
            lo = int(np.searchsorted(pj_loc, t * IBAND, side="right"))
            hi = int(np.searchsorted(pj_loc, (t + 1) * IBAND, side="left"))
            lo_u, hi_u = min(lo_u, lo), max(hi_u, hi)
        windows.append((lo_u, hi_u))
    return d, r, trk, u, tm, r_e, iord, jord, total, tuple(windows)


def _prep_inputs(event_indicator, event_time, estimate):
    (_d, _r, _trk, u, tm, r_e, iord, jord, _total,
     _windows) = _structure(event_indicator, event_time, estimate)

    in_maps = []
    for c in range(NCORES):
        iq, jh = divmod(c, NJH)
        li = np.arange(IBLK)
        isel = iord[(li // P) * IBAND + (li % P) * NIQ + iq]
        jsel = jord[jh::NJH]
        tmi = np.ascontiguousarray(
            tm[isel].astype(np.float32).reshape(IT, P).T)
        rei = np.ascontiguousarray(
            r_e[isel].astype(np.float32).reshape(IT, P).T)
        b32 = np.ascontiguousarray(
            np.concatenate([tmi, rei], axis=1)).view(np.uint8).reshape(P, -1)
        rj_b = np.ascontiguousarray(
            np.broadcast_to(r_e[jsel][None, :], (P, JW))).view(np.uint8)
        uj_b = np.ascontiguousarray(
            np.broadcast_to(u[jsel][None, :], (P, JW))).view(np.uint8)
        ones_b = np.ascontiguousarray(np.broadcast_to(
            np.array([1.0, 0.0, 0.0, 0.0], dtype=np.float16)
            .view(np.uint8)[None, :], (P, 8)))
        in_maps.append({
            "xina": np.ascontiguousarray(
                np.concatenate([rj_b, b32, ones_b], axis=1)),
            "xinb": uj_b,
        })
    return in_maps


def _tied_host(event_indicator, event_time, estimate):
    """Exact tied_risk count (see kernel_v2 docstring)."""
    d, _t, r, trk = _encode(event_indicator, event_time, estimate)

    thr = np.float32(1e-8)
    order = np.argsort(r, kind="stable")
    rs = r[order]
    lo = np.zeros(N, dtype=np.int64)
    hi = np.zeros(N, dtype=np.int64)
    p = 0
    for k in range(N):
        while np.abs(rs[k] - rs[p]) > thr:
            p += 1
        lo[k] = p
    p = N - 1
    for k in range(N - 1, -1, -1):
        while np.abs(rs[k] - rs[p]) > thr:
            p -= 1
        hi[k] = p

    cnt = hi - lo + 1
    T = int(cnt.sum())
    K = np.repeat(np.arange(N, dtype=np.int64), cnt)
    offs = np.concatenate(([0], np.cumsum(cnt)[:-1]))
    Ppos = np.arange(T, dtype=np.int64) - np.repeat(offs, cnt) + np.repeat(lo, cnt)
    i_idx = order[K]
    j_idx = order[Ppos]
    comp = d[i_idx] & (
        (trk[i_idx] < trk[j_idx])
        | ((trk[i_idx] == trk[j_idx]) & (~d[j_idx]))
    )
    return float(comp.sum())


def _finish(results, total, tied):
    conc = np.float64(0.0)
    for res in results:
        conc += res["out"].astype(np.float64).sum()
    disc = total - conc - tied
    loss = (disc + 0.5 * tied) / (disc + conc + tied + 1e-7)
    return np.asarray(1.0 - loss, dtype=np.float32)


def kernel(event_indicator, event_time, estimate):
    from concourse.bass_utils import run_bass_kernel_spmd

    st = _structure(event_indicator, event_time, estimate)
    total, windows = st[8], st[9]
    in_maps = _prep_inputs(event_indicator, event_time, estimate)
    tied = _tied_host(event_indicator, event_time, estimate)

    if _CACHE.get("windows") != windows:
        _CACHE["nc"] = _build_nc(windows)
        _CACHE["windows"] = windows
        _CACHE["primed"] = False
    nc = _CACHE["nc"]
    # Priming run: on the literal first execution after device load, PSUM
    # boot-state garbage survives under the matmul accumulation (same
    # first-execution anomaly v6 documented; DVE memset and double
    # start=True zero passes do not clear it).  Every execution >= 2 is
    # exact, so execute once and discard before the real dispatch.
    if not _CACHE.get("primed"):
        run_bass_kernel_spmd(nc, in_maps, core_ids=list(range(NCORES)))
        _CACHE["primed"] = True
    out = run_bass_kernel_spmd(nc, in_maps, core_ids=list(range(NCORES)))
    return _finish(out.results, total, tied)
